# revision 62
# baseline (speedup 1.0000x reference)
"""Causal attention (B=4, N=2048, D=1024) on 8 Trainium2 NeuronCores.

Sharding: core 2b+p handles batch b; the two cores of a batch split the KEY
tiles by parity (core p owns key tiles {p, p+2, ..., p+14}).  Each core
projects K/V AND Q only for its 8 owned tiles; the Q^T halves are exchanged
between the pair cores with a 2-rank AllGather over pair-shared HBM.  Each
core then computes unnormalized partial attention (exp-weights @ V) over its
keys for ALL 16 query tiles plus per-row exp-sums, and the host merges:
out = (O_0 + O_1) / (s_0 + s_1).  No projection work is duplicated.

Program slots are global-parity indexed: slot r*8+j is query tile 2j+r
(rank r's j-th owned tile).  L(s) = (s%8)+1 key tiles per slot; the mask on
the last processed tile comes from per-core mask data (core p sees its own
parity slots with a causal-tri diagonal mask, the other parity with either a
full pass or an all--inf pad mask).

Everything runs in bfloat16 on the PE (full rate, cheap transposes), with
f32 PSUM accumulation; partial outputs return in bf16 + f32 row-sums.
"""
import sys

sys.path.insert(0, "/opt/trn_rl_repo")

from contextlib import ExitStack

import ml_dtypes
import numpy as np

import concourse.bass as bass
import concourse.mybir as mybir
import concourse.tile as tile
from concourse import bacc
from concourse.bass_utils import run_bass_kernel_spmd
from concourse.masks import make_identity

B, N, D = 4, 2048, 1024
N_CORES = 8
N_TILES = 16         # 128-token tiles per batch
SCALE = 1.0 / 32.0   # 1/sqrt(D)
NEG = -1.0e9

F32 = mybir.dt.float32
BF16 = mybir.dt.bfloat16
BF = ml_dtypes.bfloat16

_NC_CACHE = {}
TRACE = False
LAST_EXEC_NS = None

PAIRS = [[0, 1], [2, 3], [4, 5], [6, 7]]


def _build_nc():
    nc = bacc.Bacc(None, target_bir_lowering=False, debug=False, num_devices=8)

    # All inputs partition-major so DMA descriptors are large contiguous runs.
    # x (owned tiles only), pre-transposed + per-core tile-permuted on host:
    # [partition(d%128), owned tile j, dchunk, token]
    xt = nc.declare_dram_parameter("xt", [128, 8, 8, 128], BF16, isOutput=False)
    # wq/wk: [p(d%128), echunk, dchunk, ecol]; wv: [p, dchunk, ehalf, ecol]
    wq = nc.declare_dram_parameter("wq", [128, 8, 8, 128], BF16, isOutput=False)
    wk = nc.declare_dram_parameter("wk", [128, 8, 8, 128], BF16, isOutput=False)
    wv = nc.declare_dram_parameter("wv", [128, 8, 2, 512], BF16, isOutput=False)
    # masks[:,0] for slots 0..7 (parity-0 queries), [:,1] for slots 8..15.
    # core0: [tri, zeros]; core1: [-1e9 pad, tri]
    masks = nc.declare_dram_parameter("masks", [128, 2, 128], BF16, isOutput=False)
    out_o = nc.declare_dram_parameter("out_o", [N_TILES, 128, D], BF16, isOutput=True)
    out_s = nc.declare_dram_parameter("out_s", [128, N_TILES], F32, isOutput=True)

    # Q^T exchange buffers: [p, echunk, own slot, token]; gathered adds rank
    qt_own = nc.dram_tensor("qt_own", [128, 8, 4, 2, 128], BF16, kind="Internal")
    qt_gat = nc.dram_tensor("qt_gat", [2, 128, 8, 4, 2, 128], BF16, kind="Internal")

    with tile.TileContext(nc) as tc, ExitStack() as top:
        consts = top.enter_context(tc.tile_pool(name="consts", bufs=1))
        res = top.enter_context(tc.tile_pool(name="res", bufs=1))
        xt_pool = top.enter_context(tc.tile_pool(name="xtp", bufs=1))
        qst_pool = top.enter_context(tc.tile_pool(name="qst", bufs=2))
        p_pool = top.enter_context(tc.tile_pool(name="pp", bufs=2))
        pt_pool = top.enter_context(tc.tile_pool(name="ptp", bufs=2))
        out_pool = top.enter_context(tc.tile_pool(name="op", bufs=2))
        ps = top.enter_context(tc.tile_pool(name="ps", bufs=1, space="PSUM"))

        ident_f = consts.tile([128, 128], F32)
        make_identity(nc, ident_f)
        ident = consts.tile([128, 128], BF16)
        nc.vector.tensor_copy(ident, ident_f)
        mask_sb = consts.tile([128, 2, 128], BF16)

        # SBUF residents
        QT = res.tile([128, 2, 8, 4, 2, 128], BF16)  # [p, rank, e, grp, j2, q]
        KT = res.tile([128, 8, 1024], BF16)       # [e%128, echunk, key(j*128+kk)]
        V = res.tile([128, 8, 1024], BF16)        # [token%128, tile j, e]
        wq_sb = res.tile([128, 8, 8, 128], BF16)  # [d%128, echunk, dchunk, ecol]
        wk_sb = res.tile([128, 8, 8, 128], BF16)
        wv_sb = res.tile([128, 8, 2, 512], BF16)  # [d%128, dchunk, ehalf, ecol]
        rsums = res.tile([128, 16], F32)

        def load_x(bi):
            xT = xt_pool.tile([128, 4, 8, 128], BF16, tag=f"xT{bi}", name=f"x{bi}")
            if bi == 0:
                # pair 1 in dchunk halves on gpsimd (first matmuls start
                # after 256 KB), pair 2 on scalar ahead of wq_e0
                nc.gpsimd.dma_start(out=xT[:, 0:2, 0:4, :], in_=xt[:, 0:2, 0:4, :])
                nc.gpsimd.dma_start(out=xT[:, 0:2, 4:8, :], in_=xt[:, 0:2, 4:8, :])
                nc.scalar.dma_start(out=xT[:, 2:4, :, :], in_=xt[:, 2:4, :, :])
            else:
                nc.gpsimd.dma_start(out=xT, in_=xt[:, bi * 4:bi * 4 + 4, :, :])
            return xT

        # scalar queue (fastest HWDGE): weights, ordered by consumption time;
        # gpsimd: bulk x; sync (slow): masks + qt_own spill.  wq_e0 lands in
        # dchunk halves so the first matmul starts after a 128 KB load.
        xb0 = load_x(0)
        nc.scalar.dma_start(out=wq_sb[:, 0:1, 0:4, :], in_=wq[:, 0:1, 0:4, :])
        nc.scalar.dma_start(out=wq_sb[:, 0:1, 4:8, :], in_=wq[:, 0:1, 4:8, :])
        for e in range(1, 8):
            nc.scalar.dma_start(out=wq_sb[:, e:e + 1, :, :], in_=wq[:, e:e + 1, :, :])
        xb1 = load_x(1)
        xbufs = [xb0, xb1]
        nc.scalar.dma_start(out=wk_sb[:, 0:4, :, :], in_=wk[:, 0:4, :, :])
        nc.scalar.dma_start(out=wk_sb[:, 4:8, :, :], in_=wk[:, 4:8, :, :])
        nc.sync.dma_start(out=mask_sb, in_=masks[:, :, :])
        nc.sync.dma_start(out=wv_sb[:, 0:4, :, :], in_=wv[:, 0:4, :, :])
        nc.sync.dma_start(out=wv_sb[:, 4:8, :, :], in_=wv[:, 4:8, :, :])

        def _q_store(bi, e, qps):
            qst = qst_pool.tile([128, 512], BF16, tag="qs", name=f"qs{bi}_{e}")
            nc.vector.tensor_copy(qst, qps)
            nc.gpsimd.dma_start(
                out=qt_own[:, e, bi * 2:bi * 2 + 2, :, :],
                in_=qst.rearrange("p (g j q) -> p g j q", g=2, j=2),
            )

        def proj_q(bi, xT, split_first=0):
            """Q^T for own slots bi*4 .. bi*4+3 -> qt_own DRAM (for exchange)."""
            if split_first:
                # first two e-groups phased by x arrival: pair-1 dchunk
                # halves first, then pair-2 — matmuls start on 256 KB of x
                qsp = [ps.tile([128, 512], F32, tag="acc", bufs=2, name=f"qs_{e}")
                       for e in range(2)]
                for ch in range(2):
                    for e in range(2):
                        for c in range(ch * 4, ch * 4 + 4):
                            nc.tensor.matmul(
                                qsp[e][:, 0:256],
                                wq_sb[:, e, c, :], xT[:, 0:2, c, :],
                                start=(c == 0), stop=(c == 7),
                            )
                for e in range(2):
                    for c in range(8):
                        nc.tensor.matmul(
                            qsp[e][:, 256:512],
                            wq_sb[:, e, c, :], xT[:, 2:4, c, :],
                            start=(c == 0), stop=(c == 7),
                        )
                for e in range(2):
                    _q_store(bi, e, qsp[e])
            for e in range(2 if split_first else 0, 8):
                qps = ps.tile([128, 512], F32, tag="acc", bufs=2, name=f"q{bi}_{e}")
                for c in range(8):
                    nc.tensor.matmul(
                        qps, wq_sb[:, e, c, :], xT[:, :, c, :],
                        start=(c == 0), stop=(c == 7),
                    )
                _q_store(bi, e, qps)

        def proj_k(bi, xT):
            """K^T for owned tiles bi*4 .. bi*4+3 (bi in {0,1})."""
            for e in range(8):
                kps = ps.tile([128, 512], F32, tag="acc", bufs=2, name=f"k{bi}_{e}")
                for c in range(8):
                    nc.tensor.matmul(
                        kps, wk_sb[:, e, c, :], xT[:, :, c, :],
                        start=(c == 0), stop=(c == 7),
                    )
                nc.vector.tensor_copy(KT[:, e, bi * 512:bi * 512 + 512], kps)

        def proj_v(bi, xT):
            for j in range(4):
                for eh in range(2):
                    vps = ps.tile([128, 512], F32, tag="acc", bufs=2,
                                  name=f"v{bi}_{j}_{eh}")
                    for c in range(8):
                        nc.tensor.matmul(
                            vps, xT[:, j, c, :], wv_sb[:, c, eh, :],
                            start=(c == 0), stop=(c == 7),
                        )
                    nc.vector.tensor_copy(
                        V[:, bi * 4 + j, eh * 512:eh * 512 + 512], vps
                    )

        def emit_av(prev, final=False):
            s, L, P_sb = prev
            O_ps = ps.tile([128, D], F32, tag="O", bufs=1, name=f"O{s}")
            for kt in range(L):
                ptps = ps.tile([128, 128], BF16, tag="acc", bufs=2, name=f"tp{s}_{kt}")
                nc.tensor.transpose(ptps, P_sb[:, kt * 128:(kt + 1) * 128], ident)
                pt_sb = pt_pool.tile([128, 128], BF16, tag="pt", name=f"pt{s}_{kt}")
                nc.vector.tensor_copy(pt_sb, ptps)
                for h in range(2):
                    nc.tensor.matmul(
                        O_ps[:, h * 512:(h + 1) * 512], pt_sb,
                        V[:, kt, h * 512:(h + 1) * 512],
                        start=(kt == 0), stop=(kt == L - 1),
                    )
            # out copy split across scalar+vector so neither queue (exp on
            # scalar, pt copies on vector) blocks long behind it
            out_sb = out_pool.tile([128, D], BF16, tag="osb", name=f"ou{s}")
            nc.scalar.copy(out_sb[:, 0:512], O_ps[:, 0:512])
            nc.vector.tensor_copy(out_sb[:, 512:1024], O_ps[:, 512:1024])
            if final:
                nc.sync.dma_start(out=out_o[s][:, 0:512], in_=out_sb[:, 0:512])
                nc.scalar.dma_start(out=out_o[s][:, 512:1024], in_=out_sb[:, 512:1024])
            else:
                eng = nc.sync if s % 2 == 0 else nc.scalar
                eng.dma_start(out=out_o[s][:, :], in_=out_sb)

        def do_slot(s, prev):
            L = (s % 8) + 1
            mi = s // 8
            r, j = divmod(s, 8)
            S_ps = ps.tile([128, L * 128], F32, tag="S", bufs=2, name=f"S{s}")
            ngroups = (L * 128 + 511) // 512
            for kg in range(ngroups):
                w = min(512, L * 128 - kg * 512)
                for e in range(8):
                    nc.tensor.matmul(
                        S_ps[:, kg * 512:kg * 512 + w],
                        QT[:, r, e, j // 2, j % 2, :],
                        KT[:, e, kg * 512:kg * 512 + w],
                        start=(e == 0), stop=(e == 7),
                    )
            # mask add on the tensor engine: identity @ mask accumulated onto
            # the already-closed group — no vector hop between matmuls and exp
            nc.tensor.matmul(
                S_ps[:, (L - 1) * 128:L * 128],
                ident, mask_sb[:, mi, :],
                start=False, stop=True, skip_group_check=True,
            )
            # |scores|/32 is small; exp without max-subtraction, fused row-sum
            P_sb = p_pool.tile([128, L * 128], BF16, tag="P", name=f"P{s}")
            nc.scalar.activation(
                P_sb, S_ps, mybir.ActivationFunctionType.Exp,
                bias=0.0, scale=SCALE, accum_out=rsums[:, s:s + 1],
            )
            if prev is not None:
                emit_av(prev)
            return (s, L, P_sb)

        # ---- schedule ----
        # Q^T halves exchanged pairwise, one collective per 4-slot batch so
        # the first exchange starts while the second batch still projects
        proj_q(0, xbufs[0], split_first=2)
        proj_q(1, xbufs[1])
        # 2-rank AllGather over pair HBM exchanges the Q^T halves
        nc.gpsimd.collective_compute(
            "AllGather", mybir.AluOpType.bypass,
            replica_groups=PAIRS,
            ins=[qt_own[:, :, :, :, :]],
            outs=[qt_gat[:, :, :, :, :, :]],
        )
        proj_k(0, xbufs[0])
        # read both gathered halves back (own half included — uniform program)
        nc.gpsimd.dma_start(out=QT[:, 0, :, :, :, :], in_=qt_gat[0][:, :, :, :, :])
        nc.scalar.dma_start(out=QT[:, 1, :, :, :, :], in_=qt_gat[1][:, :, :, :, :])
        proj_k(1, xbufs[1])
        proj_v(0, xbufs[0])
        proj_v(1, xbufs[1])

        # software-pipelined attention: tiny slots spread between big ones so
        # their PSUM-release bubbles hide under the big S matmul blocks
        prev = None
        for s in (7, 15, 6, 0, 14, 8, 5, 1, 13, 9, 4, 2, 12, 10, 3, 11):
            prev = do_slot(s, prev)
        nc.sync.dma_start(out=out_s[:, :], in_=rsums)
        emit_av(prev, final=True)

    nc.compile()
    return nc


def _tri_mask():
    q = np.arange(128)[:, None]
    k = np.arange(128)[None, :]
    return np.where(k <= q, 0.0, NEG).astype(np.float32)


def kernel(x, Wq, Wk, Wv):
    global LAST_EXEC_NS
    x = np.ascontiguousarray(np.asarray(x, dtype=np.float32))
    Wq = np.ascontiguousarray(np.asarray(Wq, dtype=np.float32))
    Wk = np.ascontiguousarray(np.asarray(Wk, dtype=np.float32))
    Wv = np.ascontiguousarray(np.asarray(Wv, dtype=np.float32))

    if "nc" not in _NC_CACHE:
        _NC_CACHE["nc"] = _build_nc()
    nc = _NC_CACHE["nc"]

    # host pre-transpose: x[b] (N, D) -> (p=d%128, tile, dchunk, token), bf16,
    # partition-major so each DMA descriptor covers a long contiguous run
    xt_all = np.ascontiguousarray(
        x.reshape(B, N_TILES, 128, 8, 128).transpose(0, 4, 1, 3, 2).astype(BF)
    )  # [B, p, tile, c, q]
    wq_r = np.ascontiguousarray(Wq.reshape(8, 128, 8, 128).transpose(1, 2, 0, 3).astype(BF))
    wk_r = np.ascontiguousarray(Wk.reshape(8, 128, 8, 128).transpose(1, 2, 0, 3).astype(BF))
    wv_r = np.ascontiguousarray(Wv.reshape(8, 128, 2, 512).transpose(1, 0, 2, 3).astype(BF))

    tri = _tri_mask()
    zero = np.zeros((128, 128), np.float32)
    neg = np.full((128, 128), NEG, np.float32)
    in_maps = []
    for c in range(N_CORES):
        b, p = divmod(c, 2)
        own = list(range(p, 16, 2))
        m = np.stack([tri, zero], axis=1) if p == 0 else np.stack([neg, tri], axis=1)
        in_maps.append({
            "xt": np.ascontiguousarray(xt_all[b][:, own]),
            "wq": wq_r, "wk": wk_r, "wv": wv_r,
            "masks": np.ascontiguousarray(m.astype(BF)),
        })

    res = run_bass_kernel_spmd(nc, in_maps, list(range(N_CORES)), trace=TRACE)
    LAST_EXEC_NS = res.exec_time_ns

    # host softmax-merge: out = (O_even + O_odd) / (s_even + s_odd);
    # slot s holds query tile 2*(s%8) + s//8 on every core
    Osum = np.zeros((B, N_TILES, 128, D), np.float32)
    Ssum = np.zeros((B, N_TILES, 128), np.float32)
    for c in range(N_CORES):
        b, p = divmod(c, 2)
        oo = np.asarray(res.results[c]["out_o"], dtype=np.float32)
        ss = res.results[c]["out_s"]
        for s in range(N_TILES):
            q = 2 * (s % 8) + s // 8
            Osum[b, q] += oo[s]
            Ssum[b, q] += ss[:, s]
    out = Osum / Ssum[..., None]
    return np.ascontiguousarray(out.reshape(B, N, D))


# revision 63
# speedup vs baseline: 1.0434x; 1.0434x over previous
"""Causal attention (B=4, N=2048, D=1024) on 8 Trainium2 NeuronCores.

Sharding: core 2b+p handles batch b; the two cores of a batch split the KEY
tiles by parity (core p owns key tiles {p, p+2, ..., p+14}).  Each core
projects K/V AND Q only for its 8 owned tiles; the Q^T halves are exchanged
between the pair cores with a 2-rank AllGather over pair-shared HBM.  Each
core then computes unnormalized partial attention (exp-weights @ V) over its
keys for ALL 16 query tiles plus per-row exp-sums, and the host merges:
out = (O_0 + O_1) / (s_0 + s_1).  No projection work is duplicated.

Program slots are global-parity indexed: slot r*8+j is query tile 2j+r
(rank r's j-th owned tile).  L(s) = (s%8)+1 key tiles per slot; the mask on
the last processed tile comes from per-core mask data (core p sees its own
parity slots with a causal-tri diagonal mask, the other parity with either a
full pass or an all--inf pad mask).

Everything runs in bfloat16 on the PE (full rate, cheap transposes), with
f32 PSUM accumulation; partial outputs return in bf16 + f32 row-sums.
"""
import sys

sys.path.insert(0, "/opt/trn_rl_repo")

from contextlib import ExitStack

import ml_dtypes
import numpy as np

import concourse.bass as bass
import concourse.mybir as mybir
import concourse.tile as tile
from concourse import bacc
from concourse.bass_utils import run_bass_kernel_spmd
from concourse.masks import make_identity

B, N, D = 4, 2048, 1024
N_CORES = 8
N_TILES = 16         # 128-token tiles per batch
SCALE = 1.0 / 32.0   # 1/sqrt(D)
NEG = -1.0e9

F32 = mybir.dt.float32
BF16 = mybir.dt.bfloat16
BF = ml_dtypes.bfloat16

_NC_CACHE = {}
TRACE = False
LAST_EXEC_NS = None

PAIRS = [[0, 1], [2, 3], [4, 5], [6, 7]]


def _build_nc():
    nc = bacc.Bacc(None, target_bir_lowering=False, debug=False, num_devices=8)

    # All inputs partition-major so DMA descriptors are large contiguous runs.
    # x (owned tiles only), pre-transposed + per-core tile-permuted on host:
    # [partition(d%128), owned tile j, dchunk, token]
    xt = nc.declare_dram_parameter("xt", [128, 8, 8, 128], BF16, isOutput=False)
    # wq/wk: [p(d%128), echunk, dchunk, ecol]; wv: [p, dchunk, ehalf, ecol]
    wq = nc.declare_dram_parameter("wq", [128, 8, 8, 128], BF16, isOutput=False)
    wk = nc.declare_dram_parameter("wk", [128, 8, 8, 128], BF16, isOutput=False)
    wv = nc.declare_dram_parameter("wv", [128, 8, 2, 512], BF16, isOutput=False)
    # masks[:,0] for slots 0..7 (parity-0 queries), [:,1] for slots 8..15.
    # core0: [tri, zeros]; core1: [-1e9 pad, tri]
    masks = nc.declare_dram_parameter("masks", [128, 2, 128], BF16, isOutput=False)
    out_o = nc.declare_dram_parameter("out_o", [N_TILES, 128, D], BF16, isOutput=True)
    out_s = nc.declare_dram_parameter("out_s", [128, N_TILES], F32, isOutput=True)

    # Q^T exchange buffers: [p, echunk, own slot, token]; gathered adds rank
    qt_own = nc.dram_tensor("qt_own", [128, 8, 4, 2, 128], BF16, kind="Internal")
    qt_gat = nc.dram_tensor("qt_gat", [2, 128, 8, 4, 2, 128], BF16, kind="Internal")

    with tile.TileContext(nc) as tc, ExitStack() as top:
        consts = top.enter_context(tc.tile_pool(name="consts", bufs=1))
        res = top.enter_context(tc.tile_pool(name="res", bufs=1))
        xt_pool = top.enter_context(tc.tile_pool(name="xtp", bufs=1))
        qst_pool = top.enter_context(tc.tile_pool(name="qst", bufs=2))
        p_pool = top.enter_context(tc.tile_pool(name="pp", bufs=2))
        pt_pool = top.enter_context(tc.tile_pool(name="ptp", bufs=2))
        out_pool = top.enter_context(tc.tile_pool(name="op", bufs=2))
        ps = top.enter_context(tc.tile_pool(name="ps", bufs=1, space="PSUM"))

        ident_f = consts.tile([128, 128], F32)
        make_identity(nc, ident_f)
        ident = consts.tile([128, 128], BF16)
        nc.vector.tensor_copy(ident, ident_f)
        mask_sb = consts.tile([128, 2, 128], BF16)

        # SBUF residents
        QT = res.tile([128, 2, 8, 4, 2, 128], BF16)  # [p, rank, e, grp, j2, q]
        KT = res.tile([128, 8, 1024], BF16)       # [e%128, echunk, key(j*128+kk)]
        V = res.tile([128, 8, 1024], BF16)        # [token%128, tile j, e]
        wq_sb = res.tile([128, 8, 8, 128], BF16)  # [d%128, echunk, dchunk, ecol]
        wk_sb = res.tile([128, 8, 8, 128], BF16)
        wv_sb = res.tile([128, 8, 2, 512], BF16)  # [d%128, dchunk, ehalf, ecol]
        rsums = res.tile([128, 16], F32)

        def load_x(bi):
            xT = xt_pool.tile([128, 4, 8, 128], BF16, tag=f"xT{bi}", name=f"x{bi}")
            if bi == 0:
                # pair 1 on gpsimd, pair 2 on scalar (ahead of wq_e0): all of
                # batch 0 lands before the first weight chunk does
                nc.gpsimd.dma_start(out=xT[:, 0:2, :, :], in_=xt[:, 0:2, :, :])
                nc.scalar.dma_start(out=xT[:, 2:4, :, :], in_=xt[:, 2:4, :, :])
            else:
                nc.gpsimd.dma_start(out=xT, in_=xt[:, bi * 4:bi * 4 + 4, :, :])
            return xT

        # scalar queue (fastest HWDGE): weights, ordered by consumption time;
        # gpsimd: bulk x; sync (slow): masks + qt_own spill.  wq_e0 lands in
        # dchunk halves so the first matmul starts after a 128 KB load.
        xb0 = load_x(0)
        nc.scalar.dma_start(out=wq_sb[:, 0:1, 0:4, :], in_=wq[:, 0:1, 0:4, :])
        nc.scalar.dma_start(out=wq_sb[:, 0:1, 4:8, :], in_=wq[:, 0:1, 4:8, :])
        for e in range(1, 8):
            nc.scalar.dma_start(out=wq_sb[:, e:e + 1, :, :], in_=wq[:, e:e + 1, :, :])
        xb1 = load_x(1)
        xbufs = [xb0, xb1]
        nc.scalar.dma_start(out=wk_sb[:, 0:4, :, :], in_=wk[:, 0:4, :, :])
        nc.scalar.dma_start(out=wk_sb[:, 4:8, :, :], in_=wk[:, 4:8, :, :])
        nc.sync.dma_start(out=mask_sb, in_=masks[:, :, :])
        nc.sync.dma_start(out=wv_sb[:, 0:4, :, :], in_=wv[:, 0:4, :, :])
        nc.sync.dma_start(out=wv_sb[:, 4:8, :, :], in_=wv[:, 4:8, :, :])

        def _q_store(bi, e, qps):
            qst = qst_pool.tile([128, 512], BF16, tag="qs", name=f"qs{bi}_{e}")
            nc.vector.tensor_copy(qst, qps)
            nc.gpsimd.dma_start(
                out=qt_own[:, e, bi * 2:bi * 2 + 2, :, :],
                in_=qst.rearrange("p (g j q) -> p g j q", g=2, j=2),
            )

        def proj_q(bi, xT, split_first=0):
            """Q^T for own slots bi*4 .. bi*4+3 -> qt_own DRAM (for exchange)."""
            if split_first:
                # first two e-groups run tile-pair halves: all pair-1 work
                # first, so matmuls start before the second x pair lands
                qsp = [ps.tile([128, 512], F32, tag="acc", bufs=2, name=f"qs_{e}")
                       for e in range(2)]
                for g in range(2):
                    for e in range(2):
                        for c in range(8):
                            nc.tensor.matmul(
                                qsp[e][:, g * 256:g * 256 + 256],
                                wq_sb[:, e, c, :], xT[:, g * 2:g * 2 + 2, c, :],
                                start=(c == 0), stop=(c == 7),
                            )
                for e in range(2):
                    _q_store(bi, e, qsp[e])
            for e in range(2 if split_first else 0, 8):
                qps = ps.tile([128, 512], F32, tag="acc", bufs=2, name=f"q{bi}_{e}")
                for c in range(8):
                    nc.tensor.matmul(
                        qps, wq_sb[:, e, c, :], xT[:, :, c, :],
                        start=(c == 0), stop=(c == 7),
                    )
                _q_store(bi, e, qps)

        def proj_k(bi, xT):
            """K^T for owned tiles bi*4 .. bi*4+3 (bi in {0,1})."""
            for e in range(8):
                kps = ps.tile([128, 512], F32, tag="acc", bufs=2, name=f"k{bi}_{e}")
                for c in range(8):
                    nc.tensor.matmul(
                        kps, wk_sb[:, e, c, :], xT[:, :, c, :],
                        start=(c == 0), stop=(c == 7),
                    )
                nc.vector.tensor_copy(KT[:, e, bi * 512:bi * 512 + 512], kps)

        def proj_v(bi, xT):
            for j in range(4):
                for eh in range(2):
                    vps = ps.tile([128, 512], F32, tag="acc", bufs=2,
                                  name=f"v{bi}_{j}_{eh}")
                    for c in range(8):
                        nc.tensor.matmul(
                            vps, xT[:, j, c, :], wv_sb[:, c, eh, :],
                            start=(c == 0), stop=(c == 7),
                        )
                    nc.vector.tensor_copy(
                        V[:, bi * 4 + j, eh * 512:eh * 512 + 512], vps
                    )

        def emit_av(prev, final=False):
            s, L, P_sb = prev
            O_ps = ps.tile([128, D], F32, tag="O", bufs=1, name=f"O{s}")
            for kt in range(L):
                ptps = ps.tile([128, 128], BF16, tag="acc", bufs=2, name=f"tp{s}_{kt}")
                nc.tensor.transpose(ptps, P_sb[:, kt * 128:(kt + 1) * 128], ident)
                pt_sb = pt_pool.tile([128, 128], BF16, tag="pt", name=f"pt{s}_{kt}")
                nc.vector.tensor_copy(pt_sb, ptps)
                for h in range(2):
                    nc.tensor.matmul(
                        O_ps[:, h * 512:(h + 1) * 512], pt_sb,
                        V[:, kt, h * 512:(h + 1) * 512],
                        start=(kt == 0), stop=(kt == L - 1),
                    )
            # out copy split across scalar+vector so neither queue (exp on
            # scalar, pt copies on vector) blocks long behind it
            out_sb = out_pool.tile([128, D], BF16, tag="osb", name=f"ou{s}")
            nc.scalar.copy(out_sb[:, 0:512], O_ps[:, 0:512])
            nc.vector.tensor_copy(out_sb[:, 512:1024], O_ps[:, 512:1024])
            if final:
                nc.sync.dma_start(out=out_o[s][:, 0:512], in_=out_sb[:, 0:512])
                nc.scalar.dma_start(out=out_o[s][:, 512:1024], in_=out_sb[:, 512:1024])
            else:
                eng = nc.sync if s % 2 == 0 else nc.scalar
                eng.dma_start(out=out_o[s][:, :], in_=out_sb)

        def do_slot(s, prev):
            L = (s % 8) + 1
            mi = s // 8
            r, j = divmod(s, 8)
            S_ps = ps.tile([128, L * 128], F32, tag="S", bufs=2, name=f"S{s}")
            ngroups = (L * 128 + 511) // 512
            for kg in range(ngroups):
                w = min(512, L * 128 - kg * 512)
                for e in range(8):
                    nc.tensor.matmul(
                        S_ps[:, kg * 512:kg * 512 + w],
                        QT[:, r, e, j // 2, j % 2, :],
                        KT[:, e, kg * 512:kg * 512 + w],
                        start=(e == 0), stop=(e == 7),
                    )
            # mask add on the tensor engine: identity @ mask accumulated onto
            # the already-closed group — no vector hop between matmuls and exp
            nc.tensor.matmul(
                S_ps[:, (L - 1) * 128:L * 128],
                ident, mask_sb[:, mi, :],
                start=False, stop=True, skip_group_check=True,
            )
            # |scores|/32 is small; exp without max-subtraction, fused row-sum
            P_sb = p_pool.tile([128, L * 128], BF16, tag="P", name=f"P{s}")
            nc.scalar.activation(
                P_sb, S_ps, mybir.ActivationFunctionType.Exp,
                bias=0.0, scale=SCALE, accum_out=rsums[:, s:s + 1],
            )
            if prev is not None:
                emit_av(prev)
            return (s, L, P_sb)

        # ---- schedule ----
        # Q^T halves exchanged pairwise, one collective per 4-slot batch so
        # the first exchange starts while the second batch still projects
        proj_q(0, xbufs[0], split_first=2)
        proj_q(1, xbufs[1])
        # 2-rank AllGather over pair HBM exchanges the Q^T halves
        nc.gpsimd.collective_compute(
            "AllGather", mybir.AluOpType.bypass,
            replica_groups=PAIRS,
            ins=[qt_own[:, :, :, :, :]],
            outs=[qt_gat[:, :, :, :, :, :]],
        )
        proj_k(0, xbufs[0])
        # read both gathered halves back (own half included — uniform program)
        nc.gpsimd.dma_start(out=QT[:, 0, :, :, :, :], in_=qt_gat[0][:, :, :, :, :])
        nc.scalar.dma_start(out=QT[:, 1, :, :, :, :], in_=qt_gat[1][:, :, :, :, :])
        proj_k(1, xbufs[1])
        proj_v(0, xbufs[0])
        proj_v(1, xbufs[1])

        # software-pipelined attention: tiny slots spread between big ones so
        # their PSUM-release bubbles hide under the big S matmul blocks
        prev = None
        for s in (7, 15, 6, 0, 14, 8, 5, 1, 13, 9, 4, 2, 12, 10, 3, 11):
            prev = do_slot(s, prev)
        nc.sync.dma_start(out=out_s[:, :], in_=rsums)
        emit_av(prev, final=True)

    nc.compile()
    return nc


def _tri_mask():
    q = np.arange(128)[:, None]
    k = np.arange(128)[None, :]
    return np.where(k <= q, 0.0, NEG).astype(np.float32)


def kernel(x, Wq, Wk, Wv):
    global LAST_EXEC_NS
    x = np.ascontiguousarray(np.asarray(x, dtype=np.float32))
    Wq = np.ascontiguousarray(np.asarray(Wq, dtype=np.float32))
    Wk = np.ascontiguousarray(np.asarray(Wk, dtype=np.float32))
    Wv = np.ascontiguousarray(np.asarray(Wv, dtype=np.float32))

    if "nc" not in _NC_CACHE:
        _NC_CACHE["nc"] = _build_nc()
    nc = _NC_CACHE["nc"]

    # host pre-transpose: x[b] (N, D) -> (p=d%128, tile, dchunk, token), bf16,
    # partition-major so each DMA descriptor covers a long contiguous run
    xt_all = np.ascontiguousarray(
        x.reshape(B, N_TILES, 128, 8, 128).transpose(0, 4, 1, 3, 2).astype(BF)
    )  # [B, p, tile, c, q]
    wq_r = np.ascontiguousarray(Wq.reshape(8, 128, 8, 128).transpose(1, 2, 0, 3).astype(BF))
    wk_r = np.ascontiguousarray(Wk.reshape(8, 128, 8, 128).transpose(1, 2, 0, 3).astype(BF))
    wv_r = np.ascontiguousarray(Wv.reshape(8, 128, 2, 512).transpose(1, 0, 2, 3).astype(BF))

    tri = _tri_mask()
    zero = np.zeros((128, 128), np.float32)
    neg = np.full((128, 128), NEG, np.float32)
    in_maps = []
    for c in range(N_CORES):
        b, p = divmod(c, 2)
        own = list(range(p, 16, 2))
        m = np.stack([tri, zero], axis=1) if p == 0 else np.stack([neg, tri], axis=1)
        in_maps.append({
            "xt": np.ascontiguousarray(xt_all[b][:, own]),
            "wq": wq_r, "wk": wk_r, "wv": wv_r,
            "masks": np.ascontiguousarray(m.astype(BF)),
        })

    res = run_bass_kernel_spmd(nc, in_maps, list(range(N_CORES)), trace=TRACE)
    LAST_EXEC_NS = res.exec_time_ns

    # host softmax-merge: out = (O_even + O_odd) / (s_even + s_odd);
    # slot s holds query tile 2*(s%8) + s//8 on every core
    Osum = np.zeros((B, N_TILES, 128, D), np.float32)
    Ssum = np.zeros((B, N_TILES, 128), np.float32)
    for c in range(N_CORES):
        b, p = divmod(c, 2)
        oo = np.asarray(res.results[c]["out_o"], dtype=np.float32)
        ss = res.results[c]["out_s"]
        for s in range(N_TILES):
            q = 2 * (s % 8) + s // 8
            Osum[b, q] += oo[s]
            Ssum[b, q] += ss[:, s]
    out = Osum / Ssum[..., None]
    return np.ascontiguousarray(out.reshape(B, N, D))
